# revision 1
# baseline (speedup 1.0000x reference)
"""Trainium2 Bass kernel for nn_Attention_local — v2.

Data-parallel over batch: 8 images -> 8 NeuronCores, no collectives.

Strategy vs v1: the 3x3 depthwise conv for q,k is folded into the 1x1 qkv
conv as 9 shifted matmuls in fp8e4 DoubleRow mode (K=256 packed, 0.5
cyc/col); errors wash out through l2-normalization and softmax averaging.
The v path stays bf16 (1x1 on PE, depthwise elementwise split across
DVE/Pool/Act). All intermediates bf16 in SBUF; attention QK^T also fp8
DoubleRow. No DRAM round-trips except a small v-layout staging buffer.

Layouts:
- d-order per head: d = c_local*16 + (fy*4+fx); n = h1*32+w1 (consistent
  permutation of the reference head layout; attention is equivariant).
- x uploaded zero-padded to 130x130 (origin at (1,1)) so all 9 taps are
  plain offset reads; a ones-channel (row 64 of the second K-tile) carries
  the qkv bias through the fold with exact zero-padding semantics.
"""

import numpy as np

HEADS = 4
C = 192
HW = 128
NPIX = HW * HW
PW = 130
PAD = PW * PW            # 16900
G = 32
NP = G * G               # 1024
PH = 16
DH = 48 * PH             # 768
S = 128.0                # fp8 weight scale for the qk fold (cancels in norms)
EPS = 1e-12

TAPS = [(dy, dx) for dy in (-1, 0, 1) for dx in (-1, 0, 1)]

_COMPILED = {}
_SKIP_TRANSPOSE = False


def _build(dbg=False):
    import concourse.bass as bass
    import concourse.bacc as bacc
    import concourse.mybir as mybir
    from concourse.tile import TileContext
    from concourse.masks import make_identity
    from contextlib import ExitStack

    F32 = mybir.dt.float32
    BF16 = mybir.dt.bfloat16
    FP8 = mybir.dt.float8e4
    AF = mybir.ActivationFunctionType
    ALU = mybir.AluOpType
    DR = mybir.MatmulPerfMode.DoubleRow
    AP = bass.AP

    nc = bacc.Bacc("TRN2", target_bir_lowering=False, debug=False)

    xpk_d = nc.dram_tensor("xpk", [128, 2 * PAD], FP8, kind="ExternalInput")
    xb1_d = nc.dram_tensor("xb1", [128, PAD], BF16, kind="ExternalInput")
    xb2_d = nc.dram_tensor("xb2", [65, PAD], BF16, kind="ExternalInput")
    wq8_d = nc.dram_tensor("wq8", [128, 2 * 9 * 384], FP8, kind="ExternalInput")
    wva_d = nc.dram_tensor("wva", [128, C], BF16, kind="ExternalInput")
    wvb_d = nc.dram_tensor("wvb", [65, C], BF16, kind="ExternalInput")
    wvfa_d = nc.dram_tensor("wvfa", [128, 5 * C], BF16, kind="ExternalInput")
    wvfb_d = nc.dram_tensor("wvfb", [65, 5 * C], BF16, kind="ExternalInput")
    dwv_d = nc.dram_tensor("dwv", [C, 10], F32, kind="ExternalInput")
    tpc_d = nc.dram_tensor("tpc", [C, 1], F32, kind="ExternalInput")
    pta_d = nc.dram_tensor("pta", [128, C], BF16, kind="ExternalInput")
    ptb_d = nc.dram_tensor("ptb", [65, C], BF16, kind="ExternalInput")
    ones8_d = nc.dram_tensor("ones8", [128, 8], BF16, kind="ExternalInput")
    ones16k_d = nc.dram_tensor("ones16k", [1, NPIX], BF16, kind="ExternalInput")
    y_d = nc.dram_tensor("y", [C, NPIX], BF16, kind="ExternalOutput")
    if dbg:
        natd_d = nc.dram_tensor("natd", [384, NPIX], BF16, kind="ExternalOutput")
        qtd_d = nc.dram_tensor("qtd", [128, 4 * 2 * 4 * DH], BF16, kind="ExternalOutput")
        ktd_d = nc.dram_tensor("ktd", [128, 4 * 2 * 4 * DH], BF16, kind="ExternalOutput")
        kned_d = nc.dram_tensor("kned", [128, 24], F32, kind="ExternalOutput")
        vstd_d = nc.dram_tensor("vstd", [HEADS * DH, NP], BF16, kind="ExternalOutput")
        etd_d = nc.dram_tensor("etd", [6 * 128, DH], BF16, kind="ExternalOutput")
        o1d_d = nc.dram_tensor("o1d", [128, NPIX], BF16, kind="ExternalOutput")
        o2d_d = nc.dram_tensor("o2d", [65, NPIX], BF16, kind="ExternalOutput")

    # row-run maps for the qk transpose copies.
    # chunk m covers qk rows m*128..m*128+128 (rows 0..191 q, 192..383 k).
    # run = (col0 in chunk, ccnt, tensor 0=q/1=k, dstbase = h*768 + c_l0*16)
    def chunk_runs(m):
        runs = []
        r = m * 128
        end = r + 128
        while r < end:
            tens = 0 if r < 192 else 1
            c = r if tens == 0 else r - 192
            h, cl = c // 48, c % 48
            take = min(48 - cl, (192 if tens == 0 else 384) - r, end - r)
            runs.append((r - m * 128, take, tens, h * DH + cl * 16))
            r += take
        return runs

    # norm row ranges per chunk: (row0, cnt, is_q, channel0)
    def norm_parts(m):
        parts = []
        r = m * 128
        end = r + 128
        while r < end:
            if r < 192:
                take = min(192 - r, end - r)
                parts.append((r - m * 128, take, True, r))
            else:
                take = end - r
                parts.append((r - m * 128, take, False, r - 192))
            r += take
        return parts

    with TileContext(nc) as tc:
        with ExitStack() as es_all:
            pers = es_all.enter_context(tc.tile_pool(name="pers", bufs=1))
            qt = [pers.tile([128, 2, 4 * DH], FP8, name=f"qt{g}", tag=f"qt{g}") for g in range(4)]
            kt = [pers.tile([128, 2, 4 * DH], FP8, name=f"kt{g}", tag=f"kt{g}") for g in range(4)]
            kne = pers.tile([128, 24], F32, tag="kne")
            ident = pers.tile([128, 128], BF16, tag="ident")
            make_identity(nc, ident)
            ones8 = pers.tile([128, 8], BF16, tag="ones8")
            nc.sync.dma_start(ones8[:], ones8_d.ap())

            etp = es_all.enter_context(tc.tile_pool(name="etp", bufs=12))
            psQK = es_all.enter_context(
                tc.tile_pool(name="psQK", bufs=1, space="PSUM"))

            dram = es_all.enter_context(
                tc.tile_pool(name="dram", bufs=1, space="DRAM"))
            vst_d = dram.tile([HEADS * DH, NP], BF16, tag="vst")
            ost_d = dram.tile([HEADS * DH, NP], BF16, tag="ost")

            esAB = es_all.enter_context(ExitStack())
            xpre = esAB.enter_context(tc.tile_pool(name="xpre", bufs=1))
            xb1 = xpre.tile([128, PAD], BF16, tag="xb1")

            # ================= phase A: fp8 folded q,k =================
            with ExitStack() as esA:
                wp = esA.enter_context(tc.tile_pool(name="wp", bufs=1))
                wq8 = wp.tile([128, 2, 9, 384], FP8, tag="wq8")
                nc.sync.dma_start(
                    wq8[:].rearrange("p a t m -> p (a t m)"), wq8_d.ap())
                xq = wp.tile([128, 2, PAD], FP8, tag="xq")
                for a, b in ((0, 4290), (4290, 8580), (8580, 12870),
                             (12870, PAD)):
                    nc.sync.dma_start(
                        AP(xq.tensor, a, [[2 * PAD, 128], [PAD, 2], [1, b - a]]),
                        AP(xpk_d.ap().tensor, a,
                           [[2 * PAD, 128], [PAD, 2], [1, b - a]]))
                tpcs = wp.tile([128, 2], F32, tag="tpcs")
                nc.sync.dma_start(tpcs[0:128, 0:1], tpc_d.ap()[0:128, :])
                nc.sync.dma_start(tpcs[0:64, 1:2], tpc_d.ap()[128:192, :])
                nc.sync.dma_start(xb1[:], xb1_d.ap())

                natp = esA.enter_context(tc.tile_pool(name="natp", bufs=2))
                sqp = esA.enter_context(tc.tile_pool(name="sqp", bufs=1))
                nrmp = esA.enter_context(tc.tile_pool(name="nrmp", bufs=2))
                psA = esA.enter_context(
                    tc.tile_pool(name="psA", bufs=3, space="PSUM"))
                psT = esA.enter_context(
                    tc.tile_pool(name="psT", bufs=3, space="PSUM"))

                XF = 2 * PAD   # xq free size per partition
                WF = 2 * 9 * 384
                QF = 2 * 4 * DH

                pending_transpose = []

                def run_transpose(m, nat):
                    runs = chunk_runs(m)
                    for pg in range(2):
                        for nb in range(8):
                            tp = psT.tile([128, 1024], BF16, tag="psT")
                            for pq in range(8):
                                p = pg * 8 + pq
                                nc.tensor.transpose(
                                    tp[:, pq * 128:(pq + 1) * 128],
                                    nat[:, p * NP + nb * 128:p * NP + (nb + 1) * 128],
                                    ident[:])
                            for ri, (c0, ccnt, tens, dstbase) in enumerate(runs):
                                tgt = (qt if tens == 0 else kt)[nb // 2]
                                srcc = AP(tp.tensor, c0,
                                          [[1024, 128], [128, 8], [1, ccnt]])
                                dstc = AP(tgt.tensor,
                                          (nb % 2) * 4 * DH + dstbase + pg * 8,
                                          [[QF, 128], [1, 8], [16, ccnt]])
                                if (nb + ri) % 3 == 0:
                                    nc.scalar.copy(dstc, srcc)
                                else:
                                    nc.vector.tensor_copy(dstc, srcc)

                for m in range(3):
                    nat = natp.tile([128, NPIX], BF16, tag="nat")
                    # --- folded 9-tap matmuls, DoubleRow fp8 ---
                    for t in range(32):
                        ps = psA.tile([128, 512], F32, tag="psA")
                        for tap, (dy, dx) in enumerate(TAPS):
                            rhs = AP(xq.tensor,
                                     (4 * t + 1 + dy) * PW + 1 + dx,
                                     [[XF, 128], [PAD, 2], [PW, 4], [1, 128]])
                            lhsT = AP(wq8.tensor, tap * 384 + m * 128,
                                      [[WF, 128], [9 * 384, 2], [1, 128]])
                            nc.tensor.matmul(ps[:], lhsT, rhs,
                                             start=(tap == 0), stop=(tap == 8),
                                             perf_mode=DR)
                        # PSUM (c, fy, w1, fx) -> natural (c, p=(fy,fx), t*32+w1)
                        src = AP(ps.tensor, 0,
                                 [[512, 128], [128, 4], [4, 32], [1, 4]])
                        dst = AP(nat.tensor, t * G,
                                 [[NPIX, 128], [4 * NP, 4], [1, 32], [NP, 4]])
                        if t % 2 == 0:
                            nc.scalar.copy(dst, src)
                        else:
                            nc.vector.tensor_copy(dst, src)

                    # --- norms (per-phase chains so transposes pipeline) ---
                    nrm = nrmp.tile([128, 16], F32, tag="nrm")
                    for p in range(PH):
                        sq = sqp.tile([128, NP], BF16, tag="sq")
                        nc.scalar.activation(sq[:], nat[:, p * NP:(p + 1) * NP],
                                             AF.Square,
                                             accum_out=nrm[:, p:p + 1])
                        nc.scalar.sqrt(nrm[:, p:p + 1], nrm[:, p:p + 1])
                        nc.vector.tensor_scalar_max(
                            nrm[:, p:p + 1], nrm[:, p:p + 1], EPS)
                        nc.vector.reciprocal(nrm[:, p:p + 1], nrm[:, p:p + 1])
                        for (r0, cnt, is_q, ch0) in norm_parts(m):
                            if is_q:
                                tsl = tpcs[r0:r0 + cnt, 0:1] if m == 0 \
                                    else tpcs[0:cnt, 1:2]
                                nc.vector.tensor_scalar_mul(
                                    nrm[r0:r0 + cnt, p:p + 1],
                                    nrm[r0:r0 + cnt, p:p + 1], tsl)
                                nc.vector.tensor_scalar_mul(
                                    nat[r0:r0 + cnt, p * NP:(p + 1) * NP],
                                    nat[r0:r0 + cnt, p * NP:(p + 1) * NP],
                                    nrm[r0:r0 + cnt, p:p + 1])
                    for (r0, cnt, is_q, ch0) in norm_parts(m):
                        if is_q:
                            pass
                        else:
                            # kinv -> kne staging (e = c_l*16+p, col h*6+e//128)
                            for gi in range(cnt // 8):
                                h = (ch0 + gi * 8) // 48
                                clh = (ch0 + gi * 8) % 48
                                e0 = clh * 16
                                col = h * 6 + e0 // 128
                                src = AP(nrm.tensor, (r0 + gi * 8) * 16,
                                         [[16, 8], [1, 16]])
                                nc.sync.dma_start(kne[0:128, col:col + 1], src)

                    run_transpose(m, nat)
                    if dbg:
                        nc.sync.dma_start(
                            natd_d.ap()[m * 128:(m + 1) * 128, :], nat[:])

            if dbg:
                with ExitStack() as esD:
                    dbp = esD.enter_context(tc.tile_pool(name="dbp", bufs=2))
                    for g in range(4):
                        qtb = dbp.tile([128, 2 * 4 * DH], BF16, tag="qtb")
                        nc.vector.tensor_copy(
                            qtb[:], qt[g][:].rearrange("p a d -> p (a d)"))
                        nc.sync.dma_start(
                            qtd_d.ap()[:, g * 2 * 4 * DH:(g + 1) * 2 * 4 * DH],
                            qtb[:])
                        ktb = dbp.tile([128, 2 * 4 * DH], BF16, tag="ktb")
                        nc.vector.tensor_copy(
                            ktb[:], kt[g][:].rearrange("p a d -> p (a d)"))
                        nc.sync.dma_start(
                            ktd_d.ap()[:, g * 2 * 4 * DH:(g + 1) * 2 * 4 * DH],
                            ktb[:])
                    nc.sync.dma_start(kned_d.ap(), kne[:])

            # ================= phase B: v (bf16 1x1 + elementwise dw) ======
            with ExitStack() as esB:
                wvp = esB.enter_context(tc.tile_pool(name="wvp", bufs=1))
                wva = wvp.tile([128, C], BF16, tag="wva")
                wvb = wvp.tile([65, C], BF16, tag="wvb")
                nc.sync.dma_start(wva[:], wva_d.ap())
                nc.sync.dma_start(wvb[:], wvb_d.ap())
                wvfa = wvp.tile([128, 5, C], BF16, tag="wvfa")
                wvfb = wvp.tile([65, 5, C], BF16, tag="wvfb")
                nc.sync.dma_start(wvfa[:].rearrange("p a c -> p (a c)"), wvfa_d.ap())
                nc.sync.dma_start(wvfb[:].rearrange("p a c -> p (a c)"), wvfb_d.ap())
                dv = [wvp.tile([128, 10], F32, name="dv0", tag="dv0"),
                      wvp.tile([64, 10], F32, name="dv1", tag="dv1")]
                nc.sync.dma_start(dv[0][:], dwv_d.ap()[0:128, :])
                nc.sync.dma_start(dv[1][:], dwv_d.ap()[128:192, :])
                xvp = esB.enter_context(tc.tile_pool(name="xvp", bufs=2))

                vpadp = esB.enter_context(tc.tile_pool(name="vpadp", bufs=2))
                voutp = esB.enter_context(tc.tile_pool(name="voutp", bufs=2))
                tmpp = esB.enter_context(tc.tile_pool(name="tmpp", bufs=4))
                psV = esB.enter_context(
                    tc.tile_pool(name="psV", bufs=2, space="PSUM"))

                FOLD_TAPS = (0, 1, 2, 4, 6)  # on PE (folded weights)
                DVE_TAPS = (3, 5, 7)         # DVE scalar_tensor_tensor
                ACT_TAPS = (8,)              # Act mul + DVE add

                PADH = 66 * PW               # half-image vpad (64 rows + halo)
                VOH = PH * 512               # half-image vout

                vsh = vst_d[:].rearrange("(c p) n -> c p n", p=PH)

                xb2h = {}
                for mv, cnt, H in ((0, 128, 0), (1, 64, 0), (0, 128, 1),
                                   (1, 64, 1)):
                    if H not in xb2h:
                        xb2 = xvp.tile([65, PADH], BF16, tag="xb2")
                        nc.sync.dma_start(
                            xb2[:], xb2_d.ap()[:, 64 * H * PW:(64 * H + 66) * PW])
                        xb2h[H] = xb2
                    xb2 = xb2h[H]
                    vpad = vpadp.tile([128, PADH], BF16, tag="vpad")
                    vout = voutp.tile([128, VOH], BF16, tag="vout")
                    # zero pad columns; top/bottom halo rows at image edges
                    nc.gpsimd.memset(
                        AP(vpad.tensor, 0, [[PADH, cnt], [PW, 66]]), 0.0)
                    nc.gpsimd.memset(
                        AP(vpad.tensor, 129, [[PADH, cnt], [PW, 66]]), 0.0)
                    if H == 0:
                        nc.gpsimd.memset(
                            AP(vpad.tensor, 0, [[PADH, cnt], [1, PW]]), 0.0)
                    else:
                        nc.gpsimd.memset(
                            AP(vpad.tensor, 65 * PW, [[PADH, cnt], [1, PW]]),
                            0.0)

                    for tl in range(16):
                        t = 16 * H + tl
                        # partial-fold PSUM: 5 taps with folded weights
                        pf = psV.tile([128, 512], F32, tag="psVf")
                        for i, ft in enumerate(FOLD_TAPS):
                            dy, dx = TAPS[ft]
                            rf1 = AP(xb1.tensor, (4 * t + 1 + dy) * PW + 1 + dx,
                                     [[PAD, 128], [PW, 4], [1, 128]])
                            nc.tensor.matmul(
                                pf[0:cnt, :],
                                wvfa[:, i, mv * 128:mv * 128 + cnt],
                                rf1, start=(i == 0), stop=False)
                            rf2 = AP(xb2.tensor,
                                     (4 * tl + 1 + dy) * PW + 1 + dx,
                                     [[PADH, 65], [PW, 4], [1, 128]])
                            nc.tensor.matmul(
                                pf[0:cnt, :],
                                wvfb[:, i, mv * 128:mv * 128 + cnt],
                                rf2, start=False, stop=(i == 4))
                        # pure 1x1 PSUM (vpad source for elementwise taps)
                        ps = psV.tile([128, 512], F32, tag="psV")
                        rhs1 = AP(xb1.tensor, (4 * t + 1) * PW + 1,
                                  [[PAD, 128], [PW, 4], [1, 128]])
                        nc.tensor.matmul(ps[0:cnt, :],
                                         wva[:, mv * 128:mv * 128 + cnt],
                                         rhs1, start=True, stop=False)
                        rhs2 = AP(xb2.tensor, (4 * tl + 1) * PW + 1,
                                  [[PADH, 65], [PW, 4], [1, 128]])
                        nc.tensor.matmul(ps[0:cnt, :],
                                         wvb[:, mv * 128:mv * 128 + cnt],
                                         rhs2, start=False, stop=True)
                        # interior copy -> vpad local rows 4*tl+1..4*tl+4
                        dst = AP(vpad.tensor, (4 * tl + 1) * PW + 1,
                                 [[PADH, cnt], [PW, 4], [1, 128]])
                        src = AP(ps.tensor, 0, [[512, cnt], [128, 4], [1, 128]])
                        nc.scalar.copy(dst, src)
                        # vout init = partial fold (dw bias via ones channel)
                        dsti = AP(vout.tensor, tl * G,
                                  [[VOH, cnt], [4 * 512, 4], [1, 32], [512, 4]])
                        srci = AP(pf.tensor, 0,
                                  [[512, cnt], [128, 4], [4, 32], [1, 4]])
                        nc.scalar.copy(dsti, srci)

                    # recompute halo row from the neighboring half
                    hh = 64 if H == 0 else 63       # image row to recompute
                    hrow = 65 if H == 0 else 0      # local vpad row
                    ph = psV.tile([128, 512], F32, tag="psV")
                    rh1 = AP(xb1.tensor, (hh + 1) * PW + 1,
                             [[PAD, 128], [1, 128]])
                    nc.tensor.matmul(ph[0:cnt, 0:128],
                                     wva[:, mv * 128:mv * 128 + cnt],
                                     rh1, start=True, stop=False)
                    rh2 = AP(xb2.tensor, (hh + 1 - 64 * H) * PW + 1,
                             [[PADH, 65], [1, 128]])
                    nc.tensor.matmul(ph[0:cnt, 0:128],
                                     wvb[:, mv * 128:mv * 128 + cnt],
                                     rh2, start=False, stop=True)
                    nc.scalar.copy(
                        AP(vpad.tensor, hrow * PW + 1, [[PADH, cnt], [1, 128]]),
                        ph[0:cnt, 0:128])

                    # elementwise taps, per output phase (3-dim APs)
                    for p in range(PH):
                        fy, fx = p // 4, p % 4
                        for tap, (dy, dx) in enumerate(TAPS):
                            if tap in FOLD_TAPS:
                                continue
                            out = vout[0:cnt, p * 512:(p + 1) * 512]
                            src = AP(vpad.tensor,
                                     (fy + dy + 1) * PW + fx + dx + 1,
                                     [[PADH, cnt], [4 * PW, 16], [4, 32]])
                            w = dv[mv][0:cnt, tap:tap + 1]
                            if tap in DVE_TAPS:
                                nc.vector.scalar_tensor_tensor(
                                    out, src, w, out, op0=ALU.mult, op1=ALU.add)
                            else:
                                tmp = tmpp.tile([128, 512], BF16, tag="tmp")
                                nc.gpsimd.tensor_scalar_mul(tmp[0:cnt, :], src, w)
                                nc.vector.tensor_tensor(out, out, tmp[0:cnt, :],
                                                        op=ALU.add)

                    # vout-half -> vst rows (c*16+p), col half
                    nc.sync.dma_start(
                        vsh[mv * 128:mv * 128 + cnt, :, H * 512:(H + 1) * 512],
                        vout[0:cnt, :].rearrange("c (p n) -> c p n", p=PH))

            esAB.close()

            # QK^T + exp for all heads — PE/Act fill the B dw-drain window
            ets_h = []
            for h in range(HEADS):
                ets = []
                for ec in range(6):
                    pa = psQK.tile([128, DH], F32, tag="psQK")
                    for g in range(4):
                        lhsT = kt[g][:, :, h * DH + ec * 128:
                                     h * DH + (ec + 1) * 128]
                        nc.tensor.matmul(
                            pa[:, 0:512], lhsT,
                            qt[g][:, :, h * DH:h * DH + 512],
                            start=(g == 0), stop=(g == 3), perf_mode=DR)
                        nc.tensor.matmul(
                            pa[:, 512:DH], lhsT,
                            qt[g][:, :, h * DH + 512:h * DH + DH],
                            start=(g == 0), stop=(g == 3), perf_mode=DR)
                    et = etp.tile([128, DH], BF16, tag="et")
                    nc.scalar.activation(
                        et[:], pa[:], AF.Exp,
                        scale=kne[:, h * 6 + ec:h * 6 + ec + 1])
                    ets.append(et)
                ets_h.append(ets)

            # ================= phase C: attention + proj =================
            with ExitStack() as esC:
                orp = esC.enter_context(tc.tile_pool(name="orp", bufs=1))
                o1 = orp.tile([128, NPIX], BF16, tag="o1")
                o2 = orp.tile([65, NPIX], BF16, tag="o2")
                nc.sync.dma_start(o2[64:65, :], ones16k_d.ap())

                with ExitStack() as esC1:
                    vrp = esC1.enter_context(tc.tile_pool(name="vrp", bufs=1))
                    vr = [vrp.tile([128, NP], BF16, name=f"vr{j}", tag=f"vr{j}")
                          for j in range(24)]
                    for j in range(24):
                        nc.sync.dma_start(
                            vr[j][:], vst_d[j * 128:(j + 1) * 128, :])
                    odp = esC1.enter_context(tc.tile_pool(name="odp", bufs=3))
                    zp = esC1.enter_context(tc.tile_pool(name="zp", bufs=4))
                    psAV = esC1.enter_context(
                        tc.tile_pool(name="psAV", bufs=2, space="PSUM"))

                    for h in range(HEADS):
                        ets = ets_h[h]
                        for dc in range(6):
                            po = psAV.tile([128, 1536], F32, tag="psAV")
                            for ec in range(6):
                                st, sp = ec == 0, ec == 5
                                lhsT = ets[ec][:, dc * 128:(dc + 1) * 128]
                                v = vr[h * 6 + ec]
                                nc.tensor.matmul(po[:, 0:512], lhsT,
                                                 v[:, 0:512], start=st, stop=sp)
                                nc.tensor.matmul(po[:, 512:1024], lhsT,
                                                 v[:, 512:1024], start=st, stop=sp)
                                nc.tensor.matmul(po[:, 1024:1032], lhsT,
                                                 ones8[:], start=st, stop=sp)
                            zr = zp.tile([128, 1], F32, tag="zr")
                            nc.vector.tensor_scalar_add(zr[:], po[:, 1024:1025], 1.0)
                            nc.vector.reciprocal(zr[:], zr[:])
                            ot = odp.tile([128, NP], BF16, tag="ot")
                            nc.scalar.mul(ot[:], po[:, 0:1024], zr[:])
                            # stage out rows d = h*768+dc*128.. to DRAM
                            nc.sync.dma_start(
                                ost_d[h * DH + dc * 128:h * DH + (dc + 1) * 128, :],
                                ot[:])
                        if dbg and h == 0:
                            for ec in range(6):
                                nc.sync.dma_start(
                                    etd_d.ap()[ec * 128:(ec + 1) * 128, :],
                                    ets[ec][:])
                        # gather this head's out rows into orep as soon as done
                        ostv = ost_d[:].rearrange("(c q) n -> c (q n)", q=PH)
                        c0, c1 = h * 48, h * 48 + 48
                        if c1 <= 128:
                            nc.sync.dma_start(o1[c0:c1, :], ostv[c0:c1, :])
                        elif c0 >= 128:
                            nc.sync.dma_start(o2[c0 - 128:c1 - 128, :],
                                              ostv[c0:c1, :])
                        else:
                            nc.sync.dma_start(o1[c0:128, :], ostv[c0:128, :])
                            nc.sync.dma_start(o2[0:c1 - 128, :], ostv[128:c1, :])


                if dbg:
                    for j in range(24):
                        nc.sync.dma_start(
                            vstd_d.ap()[j * 128:(j + 1) * 128, :],
                            vst_d[j * 128:(j + 1) * 128, :])
                    nc.sync.dma_start(o1d_d.ap(), o1[:])
                    nc.sync.dma_start(o2d_d.ap(), o2[:])

                with ExitStack() as esC2:
                    pwp = esC2.enter_context(tc.tile_pool(name="pwp", bufs=1))
                    pta = pwp.tile([128, C], BF16, tag="pta")
                    ptb = pwp.tile([65, C], BF16, tag="ptb")
                    nc.sync.dma_start(pta[:], pta_d.ap())
                    nc.sync.dma_start(ptb[:], ptb_d.ap())
                    y1 = pwp.tile([128, NPIX], BF16, tag="y1")
                    y2 = pwp.tile([64, NPIX], BF16, tag="y2")
                    psY = esC2.enter_context(
                        tc.tile_pool(name="psY", bufs=3, space="PSUM"))

                    for t in range(32):
                        for mo, cnt, yt in ((0, 128, y1), (1, 64, y2)):
                            ps = psY.tile([128, 512], F32, tag="psY")
                            rhs1 = AP(o1.tensor, t * G,
                                      [[NPIX, 128], [4 * NP, 4], [1, 32], [NP, 4]])
                            nc.tensor.matmul(ps[0:cnt, :],
                                             pta[:, mo * 128:mo * 128 + cnt],
                                             rhs1, start=True, stop=False)
                            rhs2 = AP(o2.tensor, t * G,
                                      [[NPIX, 65], [4 * NP, 4], [1, 32], [NP, 4]])
                            nc.tensor.matmul(ps[0:cnt, :],
                                             ptb[:, mo * 128:mo * 128 + cnt],
                                             rhs2, start=False, stop=True)
                            if t % 2 == 0:
                                nc.scalar.copy(yt[0:cnt, t * 512:(t + 1) * 512],
                                               ps[0:cnt, :])
                            else:
                                nc.vector.tensor_copy(
                                    yt[0:cnt, t * 512:(t + 1) * 512], ps[0:cnt, :])
                    for q0 in range(0, 32, 8):
                        cs = slice(q0 * 512, (q0 + 8) * 512)
                        nc.sync.dma_start(y_d.ap()[0:128, cs], y1[:, cs])
                        nc.sync.dma_start(y_d.ap()[128:192, cs], y2[:, cs])

    nc.compile()
    return nc


def _prep_common(qkv_w, qkv_b, dw_w, dw_b, proj_w, proj_b, temp):
    import ml_dtypes
    FP8 = ml_dtypes.float8_e4m3
    BF16 = ml_dtypes.bfloat16

    dw9 = dw_w.reshape(576, 9)
    # folded qk weights, fp8, scaled by S
    wq8 = np.zeros((128, 2, 9, 384), np.float32)
    for j in range(2):
        for t in range(9):
            w = dw9[:384, t:t + 1] * qkv_w[:384, :]     # [384, 192]
            nch = 128 if j == 0 else 64
            wq8[:nch, j, t, :] = w[:, j * 128:j * 128 + nch].T
        wq8[64, 1, :, :] = (dw9[:384, :] * qkv_b[:384, None]).T
    wq8[64, 1, 4, :] += dw_b[:384]
    wq8 = (S * wq8).astype(FP8)

    wva = np.ascontiguousarray(qkv_w[384:, 0:128].T).astype(BF16)
    wvb = np.zeros((65, C), np.float32)
    wvb[0:64] = qkv_w[384:, 128:192].T
    wvb[64] = qkv_b[384:]
    wvb = wvb.astype(BF16)

    # folded v weights for PE taps (center carries dw bias)
    FT = (0, 1, 2, 4, 6)
    wvfa = np.zeros((128, 5, C), np.float32)
    wvfb = np.zeros((65, 5, C), np.float32)
    for i, t in enumerate(FT):
        w = dw9[384:, t:t + 1] * qkv_w[384:, :]      # [192, 192]
        wvfa[:, i, :] = w[:, 0:128].T
        wvfb[0:64, i, :] = w[:, 128:192].T
        wvfb[64, i, :] = dw9[384:, t] * qkv_b[384:] + (dw_b[384:] if t == 4 else 0)
    wvfa = wvfa.astype(BF16)
    wvfb = wvfb.astype(BF16)
    # all-9-tap fold weights for the last chunk (rows 128..191)
    wv9a = np.zeros((128, 9, C), np.float32)
    wv9b = np.zeros((65, 9, C), np.float32)
    for t in range(9):
        w = dw9[384:, t:t + 1] * qkv_w[384:, :]
        wv9a[:, t, :] = w[:, 0:128].T
        wv9b[0:64, t, :] = w[:, 128:192].T
        wv9b[64, t, :] = dw9[384:, t] * qkv_b[384:] + (dw_b[384:] if t == 4 else 0)
    wv9a = wv9a.astype(BF16)
    wv9b = wv9b.astype(BF16)

    dwv = np.zeros((C, 10), np.float32)
    dwv[:, 0:9] = dw9[384:]
    dwv[:, 9] = dw_b[384:]

    pta = np.ascontiguousarray(proj_w[:, 0:128].T).astype(BF16)
    ptb = np.zeros((65, C), np.float32)
    ptb[0:64] = proj_w[:, 128:192].T
    ptb[64] = proj_b
    ptb = ptb.astype(BF16)

    tpc = np.repeat(temp, 48).reshape(C, 1).astype(np.float32)

    return {
        "wq8": np.ascontiguousarray(wq8.reshape(128, 2 * 9 * 384)),
        "wva": wva,
        "wvb": wvb,
        "wvfa": np.ascontiguousarray(wvfa.reshape(128, 5 * C)),
        "wvfb": np.ascontiguousarray(wvfb.reshape(65, 5 * C)),
        "wv9a": np.ascontiguousarray(wv9a.reshape(128, 9 * C)),
        "wv9b": np.ascontiguousarray(wv9b.reshape(65, 9 * C)),
        "dwv": dwv,
        "tpc": tpc,
        "pta": pta,
        "ptb": ptb,
        "ones8": np.ones((128, 8), BF16),
        "ones16k": np.ones((1, NPIX), BF16),
    }


def _prep_x(xb):
    import ml_dtypes
    FP8 = ml_dtypes.float8_e4m3
    BF16 = ml_dtypes.bfloat16
    xp = np.zeros((C, PW, PW), np.float32)
    xp[:, 1:129, 1:129] = xb
    ones = np.zeros((PW, PW), np.float32)
    ones[1:129, 1:129] = 1.0

    xpk = np.zeros((128, 2, PAD), np.float32)
    xpk[:, 0, :] = xp[0:128].reshape(128, PAD)
    xpk[0:64, 1, :] = xp[128:192].reshape(64, PAD)
    xpk[64, 1, :] = ones.reshape(PAD)

    xb2 = np.zeros((65, PAD), np.float32)
    xb2[0:64] = xp[128:192].reshape(64, PAD)
    xb2[64] = ones.reshape(PAD)

    return {
        "xpk": np.ascontiguousarray(xpk.reshape(128, 2 * PAD)).astype(FP8),
        "xb1": xp[0:128].reshape(128, PAD).astype(BF16),
        "xb2": xb2.astype(BF16),
    }


def kernel(**inputs):
    import concourse.bass_utils as bu

    x = np.asarray(inputs["x"], np.float32)
    qkv_w = np.asarray(inputs["qkv_w"], np.float32)
    qkv_b = np.asarray(inputs["qkv_b"], np.float32)
    dw_w = np.asarray(inputs["dw_w"], np.float32)
    dw_b = np.asarray(inputs["dw_b"], np.float32)
    proj_w = np.asarray(inputs["proj_w"], np.float32)
    proj_b = np.asarray(inputs["proj_b"], np.float32)
    temp = np.asarray(inputs["temperature"], np.float32).reshape(HEADS)

    if "nc" not in _COMPILED:
        _COMPILED["nc"] = _build()
    nc = _COMPILED["nc"]

    common = _prep_common(qkv_w, qkv_b, dw_w, dw_b, proj_w, proj_b, temp)
    in_maps = [{**_prep_x(x[b]), **common} for b in range(x.shape[0])]
    res = bu.run_bass_kernel_spmd(nc, in_maps, core_ids=list(range(len(in_maps))))
    out = np.stack([
        r["y"].astype(np.float32).reshape(C, HW, HW) for r in res.results])
    return out

